# revision 7
# baseline (speedup 1.0000x reference)
"""LinearAttention Trainium2 kernel: data-parallel over batch on 8 cores.

Math (validated vs reference):
  Wq' = per-head Wq @ P (folds feature map into the Q projection), same for K.
  QkT = relu(Wq'^T @ q^T + bq')            [HF, tokens]  (transposed activations)
  Ksum[hf, b] = sum_s relu(Wk'^T k^T)      (ACT accum_out per batch strip)
  U^T[c, b, h] = sum_d WvT[hd, c] Ksum[hd, b]   (folds the V projection away)
  SrowT[v, h]  = sum_c value[b, v, c] U^T[c, b, h]
  Z = per-head column sums of QkT (indicator matmuls), Zrec = 1/(Z + 257e-8)
  outT = (QkT + eps) * Srow * Zrec ; finalT = Wo^T @ outT + bo
"""
import numpy as np
import ml_dtypes

B, S, D, H = 64, 256, 2048, 8
DK = D // H
F = 256
EPS = 1e-8
NCORES = 8
BL = B // NCORES          # 8 batches per core
M = BL * S                # 2048 tokens per core
KT = D // 128             # 16 k-tiles


def _build():
    import concourse.bass as bass
    import concourse.mybir as mybir
    import concourse.tile as tile_mod
    from concourse.vector_clock import ScopedClock

    # ---- workaround: this walrus build allows ONE sync wait per instruction.
    # Split multi-wait instructions by inserting same-engine NoOp carriers.
    if not getattr(tile_mod, "_onewait_patched", False):
        _orig_add = tile_mod.TileContext._add_instruction

        def _patched_add(self, inst):
            si = inst.sync_info
            if si is not None and si.on_wait is not None and len(si.on_wait) > 1:
                waits = list(si.on_wait)
                for w in waits[:-1]:
                    nop = mybir.InstNoOp(name=self.nc.get_next_instruction_name())
                    nop.engine = inst.engine
                    nop.sync_info = mybir.SyncInfo(on_wait=[w], on_update=[])
                    _orig_add(self, nop)
                inst.sync_info = mybir.SyncInfo(
                    on_wait=[waits[-1]], on_update=list(si.on_update)
                )
            _orig_add(self, inst)

        def _patched_drain(self, tick_clock, wait_clock):
            gc = tick_clock.global_clock
            items = gc.items() if hasattr(gc, "items") else [(None, gc)]
            for scope, vc in items:
                for proc in range(len(vc)):
                    t = vc[proc]
                    if t > 0:
                        nop = self.nc.sync.nop()
                        req = ScopedClock()
                        req.require_at_least(scope, proc, t)
                        wait_clock.add_sem_waits(nop.ins, req)
            self.nc.sync.drain()
            self.nc.all_engine_barrier()
            popped = self.nc._tile_sem_poison_stack.pop()
            assert popped is self._sem_poison
            self.nc.clear_and_free_semaphores(list(self.sems.allocated().values()))
            self.nc.all_engine_barrier()

        tile_mod.TileContext._add_instruction = _patched_add
        tile_mod.TileContext._drain_and_barrier = _patched_drain
        tile_mod._onewait_patched = True

    f32 = mybir.dt.float32
    bf16 = mybir.dt.bfloat16
    Relu = mybir.ActivationFunctionType.Relu
    Alu = mybir.AluOpType

    nc = bass.Bass()
    xq = nc.declare_dram_parameter("xq", [D, M], bf16, isOutput=False)   # query^T
    xk = nc.declare_dram_parameter("xk", [D, M], bf16, isOutput=False)   # key^T
    xv = nc.declare_dram_parameter("xv", [D, M], bf16, isOutput=False)   # value^T
    wqp = nc.declare_dram_parameter("wqp", [D, D], bf16, isOutput=False)  # Wq@P folded
    wkp = nc.declare_dram_parameter("wkp", [D, D], bf16, isOutput=False)
    wvt = nc.declare_dram_parameter("wvt", [D, D], bf16, isOutput=False)  # Wv^T
    wo = nc.declare_dram_parameter("wo", [D, D], bf16, isOutput=False)    # Wo natural
    bqp = nc.declare_dram_parameter("bqp", [D], f32, isOutput=False)
    bkp = nc.declare_dram_parameter("bkp", [D], f32, isOutput=False)
    bob = nc.declare_dram_parameter("bob", [D], f32, isOutput=False)
    fin = nc.declare_dram_parameter("fin", [D, M], f32, isOutput=True)    # final^T

    with tile_mod.TileContext(nc) as tc:
        with (
            nc.allow_low_precision(reason="bf16 pipeline by design"),
            tc.tile_pool(name="wpool", bufs=1) as wpool,
            tc.tile_pool(name="xstrip", bufs=2) as xpool,
            tc.tile_pool(name="persist", bufs=1) as ppool,
            tc.tile_pool(name="scratch", bufs=2) as spool,
            tc.tile_pool(name="qkpool", bufs=1) as qkpool,
            tc.tile_pool(name="psbig", bufs=3, space="PSUM") as psbig,
            tc.tile_pool(name="pssmall", bufs=2, space="PSUM") as pssmall,
        ):
            # constants / biases
            bq_sb = ppool.tile([128, KT], f32, tag="bq")
            bk_sb = ppool.tile([128, KT], f32, tag="bk")
            bo_sb = ppool.tile([128, KT], f32, tag="bo")
            nc.sync.dma_start(bq_sb[:], bqp.rearrange("(t p) -> p t", p=128))
            nc.sync.dma_start(bk_sb[:], bkp.rearrange("(t p) -> p t", p=128))
            nc.sync.dma_start(bo_sb[:], bob.rearrange("(t p) -> p t", p=128))
            ind = ppool.tile([128, 17], bf16, tag="ind")
            nc.vector.memset(ind[:], 0.0)
            nc.vector.memset(ind[:, 8:9], 1.0)
            ones_row = ppool.tile([1, 128], bf16, tag="ones")
            nc.vector.memset(ones_row[:], 1.0)

            ksum = ppool.tile([128, KT, 8], f32, tag="ksum")
            ksum_bf = ppool.tile([128, KT, 8], bf16, tag="ksumbf")
            ut_sb = ppool.tile([128, KT, 64], bf16, tag="ut")
            srow = ppool.tile([128, 2, 8, 8], f32, tag="srow")
            outT = ppool.tile([128, KT, M], bf16, tag="outT")     # 8.4MB

            # ---------------- phase B: Kk -> Ksum ----------------
            wk = wpool.tile([128, KT, D], bf16, tag="W")
            for i in range(4):
                nc.sync.dma_start(
                    wk[:, 4 * i:4 * (i + 1), :],
                    wkp.rearrange("(t p) m -> p t m", p=128)[:, 4 * i:4 * (i + 1), :])
            for b in range(BL):
                xs = xpool.tile([128, KT, S], bf16, tag="xs")
                nc.sync.dma_start(
                    xs[:], xk.rearrange("(t p) m -> p t m", p=128)[:, :, b * S:(b + 1) * S])
                for t in range(KT):
                    ps = psbig.tile([128, S], f32, tag="big")
                    for k in range(KT):
                        nc.tensor.matmul(ps[:], wk[:, k, t * 128:(t + 1) * 128],
                                         xs[:, k, :], start=(k == 0), stop=(k == KT - 1))
                    scr = spool.tile([128, S], bf16, tag="scr")
                    nc.scalar.activation(scr[:], ps[:], Relu,
                                         bias=bk_sb[:, t:t + 1],
                                         accum_out=ksum[:, t, b:b + 1])
            nc.vector.tensor_scalar(ksum_bf[:], ksum[:], S * EPS, None, Alu.add)

            # ---------------- U^T then Srow^T ----------------
            wv = wpool.tile([128, KT, D], bf16, tag="W")
            for i in range(4):
                nc.sync.dma_start(
                    wv[:, 4 * i:4 * (i + 1), :],
                    wvt.rearrange("(t p) m -> p t m", p=128)[:, 4 * i:4 * (i + 1), :])
            for ct in range(KT):
                psu = pssmall.tile([128, 64], f32, tag="small")
                for h in range(H):
                    for j in range(2):
                        t = 2 * h + j
                        nc.tensor.matmul(psu[:, h * 8:(h + 1) * 8],
                                         wv[:, t, ct * 128:(ct + 1) * 128],
                                         ksum_bf[:, t, :],
                                         start=(j == 0), stop=(j == 1))
                nc.vector.tensor_copy(ut_sb[:, ct, :], psu[:])
            for b in range(BL):
                xs = xpool.tile([128, KT, S], bf16, tag="xs")
                nc.sync.dma_start(
                    xs[:], xv.rearrange("(t p) m -> p t m", p=128)[:, :, b * S:(b + 1) * S])
                for vch in range(2):
                    pss = pssmall.tile([128, 8], f32, tag="small")
                    for ct in range(KT):
                        nc.tensor.matmul(pss[:], xs[:, ct, vch * 128:(vch + 1) * 128],
                                         ut_sb[:, ct, b::8],
                                         start=(ct == 0), stop=(ct == KT - 1))
                    nc.vector.tensor_copy(srow[:, vch, b, :], pss[:])

            # ---------------- phase A: Qk -> outT ----------------
            wq = wpool.tile([128, KT, D], bf16, tag="W")
            for i in range(4):
                nc.sync.dma_start(
                    wq[:, 4 * i:4 * (i + 1), :],
                    wqp.rearrange("(t p) m -> p t m", p=128)[:, 4 * i:4 * (i + 1), :])
            for n in range(8):
                NW = 256
                xs = xpool.tile([128, KT, NW], bf16, tag="xs")
                nc.sync.dma_start(
                    xs[:], xq.rearrange("(t p) m -> p t m", p=128)[:, :, n * NW:(n + 1) * NW])
                qk = qkpool.tile([128, KT, NW], bf16, tag="qk")
                for t in range(KT):
                    ps = psbig.tile([128, NW], f32, tag="big")
                    for k in range(KT):
                        nc.tensor.matmul(ps[:], wq[:, k, t * 128:(t + 1) * 128],
                                         xs[:, k, :], start=(k == 0), stop=(k == KT - 1))
                    nc.scalar.activation(qk[:, t, :], ps[:], Relu,
                                         bias=bq_sb[:, t:t + 1])
                # Z per head: ones-column matmuls -> [1, tokens] per head
                zbfs = []
                for h in range(H):
                    psz = pssmall.tile([1, NW], f32, tag="small")
                    for j in range(2):
                        nc.tensor.matmul(psz[:], ind[:, 8:9], qk[:, 2 * h + j, :],
                                         start=(j == 0), stop=(j == 1))
                    ztmp = spool.tile([1, NW], f32, tag="ztmp")
                    nc.vector.tensor_scalar(ztmp[:], psz[:], F * EPS + EPS, None, Alu.add)
                    zbf = spool.tile([1, NW], bf16, tag=f"zbf{h}")
                    nc.vector.reciprocal(zbf[:], ztmp[:])
                    zbfs.append(zbf)
                for t in range(KT):
                    h, fh = t // 2, t % 2
                    psb = psbig.tile([128, NW], f32, tag="big")
                    nc.tensor.matmul(psb[:], ones_row[:], zbfs[h][:],
                                     start=True, stop=True)
                    tmp = spool.tile([128, NW], f32, tag="tmp")
                    nc.vector.tensor_scalar(
                        tmp[:], qk[:, t, :],
                        EPS, srow[:, fh, n, h:h + 1], Alu.add, Alu.mult)
                    nc.vector.tensor_tensor(outT[:, t, n * NW:(n + 1) * NW],
                                            tmp[:], psb[:], Alu.mult)

            # ---------------- phase D: final = Wo^T @ outT + bo ----------------
            wos = wpool.tile([128, KT, D], bf16, tag="W")
            for i in range(4):
                nc.sync.dma_start(
                    wos[:, 4 * i:4 * (i + 1), :],
                    wo.rearrange("(t p) m -> p t m", p=128)[:, 4 * i:4 * (i + 1), :])
            for n in range(4):
                NW = 512
                for m in range(KT):
                    ps = psbig.tile([128, NW], f32, tag="big")
                    for k in range(KT):
                        nc.tensor.matmul(ps[:], wos[:, k, m * 128:(m + 1) * 128],
                                         outT[:, k, n * NW:(n + 1) * NW],
                                         start=(k == 0), stop=(k == KT - 1))
                    fo = spool.tile([128, NW], f32, tag="fo")
                    nc.vector.tensor_scalar(fo[:], ps[:], bo_sb[:, m:m + 1], None, Alu.add)
                    nc.sync.dma_start(fin[m * 128:(m + 1) * 128, n * NW:(n + 1) * NW], fo[:])
    return nc


_NC = None


def kernel(query, key, value, Wq, bq, Wk, bk, Wv, bv, Wo, bo, random_proj):
    global _NC
    from concourse.bass_utils import run_bass_kernel_spmd

    bf = ml_dtypes.bfloat16
    # host-side weight folding (fp32)
    Wq4 = Wq.reshape(D, H, DK)
    Wqp = np.einsum('dhk,kf->dhf', Wq4, random_proj).reshape(D, D)
    bqp = (bq.reshape(H, DK) @ random_proj).reshape(D).astype(np.float32)
    Wk4 = Wk.reshape(D, H, DK)
    Wkp = np.einsum('dhk,kf->dhf', Wk4, random_proj).reshape(D, D)
    bkp = (bk.reshape(H, DK) @ random_proj).reshape(D).astype(np.float32)

    Wqp_b = np.ascontiguousarray(Wqp).astype(bf)
    Wkp_b = np.ascontiguousarray(Wkp).astype(bf)
    WvT_b = np.ascontiguousarray(Wv.T).astype(bf)
    Wo_b = np.ascontiguousarray(Wo).astype(bf)
    bo_f = bo.astype(np.float32)

    if _NC is None:
        _NC = _build()

    in_maps = []
    for c in range(NCORES):
        sl = slice(c * BL, (c + 1) * BL)
        xqT = np.ascontiguousarray(query[sl].reshape(M, D).T).astype(bf)
        xkT = np.ascontiguousarray(key[sl].reshape(M, D).T).astype(bf)
        xvT = np.ascontiguousarray(value[sl].reshape(M, D).T).astype(bf)
        in_maps.append({
            "xq": xqT, "xk": xkT, "xv": xvT,
            "wqp": Wqp_b, "wkp": Wkp_b, "wvt": WvT_b, "wo": Wo_b,
            "bqp": bqp, "bkp": bkp, "bob": bo_f,
        })

    res = run_bass_kernel_spmd(_NC, in_maps, list(range(NCORES)))
    out = np.empty((B, S, D), dtype=np.float32)
    for c in range(NCORES):
        finT = res.results[c]["fin"]                      # [D, M]
        out[c * BL:(c + 1) * BL] = finT.T.reshape(BL, S, D)
    kernel._last_in_maps = in_maps
    return out


def run_traced(inputs):
    """Re-run with NTFF tracing; returns exec_time_ns (or None)."""
    from concourse.bass_utils import run_bass_kernel_spmd
    if not hasattr(kernel, "_last_in_maps"):
        kernel(**inputs)
    res = run_bass_kernel_spmd(_NC, kernel._last_in_maps, list(range(NCORES)),
                               trace=True)
    run_traced._last = res
    return res.exec_time_ns
